# revision 18
# baseline (speedup 1.0000x reference)
"""GRU-decoder kernel for 8 Trainium2 NeuronCores.

Math (all 127 output steps are identical -- see the reference):
    x0   = relu(emb[input[:,0]])                       [B,H]
    h0   = einsum('blh,l->bh', hidden, bridge_w) + bb  [B,H]
    gi   = x0 @ w_ih.T + b_ih ; gh = h0 @ w_hh.T + b_hh
    r,z  = sigmoid(...) ; n = tanh(in + r*hn)
    h1   = (1-z)*n + z*h0
    logp = log_softmax(h1 @ proj_w.T + proj_b)         [B,V]
    out  = broadcast(logp, [B, L-1, V])

Sharding: everything h-sliced.  Core c owns hidden-dim slice
[c*128,(c+1)*128): it computes that slice of h0 exactly (bridge contracts
over L), AllGathers h0 (4 KB), computes the r/z/n gate rows for its slice
exactly (full-H contraction, bf16), forms its h1 slice, and AllGathers h1
(2 KB, fp8).  The projection is vocab-sharded: each core owns V/8 rows of
proj_w stored fp8 (scaled x32) and computes logits with DoubleRow fp8
matmuls (h1 scaled x8, so PSUM = 256*logits).  Softmax needs no
max-subtraction (logits are O(1)); per-core sum(exp) is AllGathered (64 B)
for the global log-sum-exp.

Queueing: bulk weight streams ride the Sync-engine HWDGE FIFO; the
collective chain (payload in / AllGather / gather-back) rides the GpSimd
SWDGE queue so it never waits behind bulk transfers (HWDGE completions are
FIFO per engine).  Gate tanh is computed as 2*sigmoid(2x)-1 so the scalar
engine never swaps activation tables on the critical path; Exp/Ln tables
are preloaded with dummy ops during collective waits.

Contraction index maps (so every gather-back is one contiguous DMA):
  gates:      k = 8*kp + kc   (kp = partition, kc = chunk 0..7)
  projection: k = 8*p + 2*q + jj  (q = DoubleRow pair, jj = subtile)
Host-side packing matches these maps.
"""

import os
import numpy as np
import ml_dtypes

import concourse.bass as bass
import concourse.tile as tile
from concourse import bacc, mybir
from concourse.bass_utils import run_bass_kernel_spmd

B, L, H, V = 16, 128, 1024, 50257
NC = 8
VC = 6656                # per-core vocab shard (13*512); 8*VC = 53248 >= V
HC = 128                 # per-core hidden-dim slice
KC = 8                   # k-chunks of 128 over H
NPAIR = 4                # fp8 DoubleRow k-pairs (256-contraction each)
N_CH = VC // 512         # 13 psum chunks per core

USE_FP8 = True
DEBUG = os.environ.get("KDBG") == "1"
W_SCALE = 32.0           # proj_w pre-scale (host, exact power of 2)
H_SCALE = 8.0            # h1 pre-scale (device, exact power of 2)
INV_SCALE = 1.0 / (W_SCALE * H_SCALE)
PAD_PB = -10000.0        # bias for padded vocab rows -> exp()=0, masked out

f32 = mybir.dt.float32
bf16 = mybir.dt.bfloat16
f8 = mybir.dt.float8e4
FX = mybir.ActivationFunctionType
AX = mybir.AxisListType
ALU = mybir.AluOpType
DR = mybir.MatmulPerfMode.DoubleRow

VG_W = [2048, 2048, 2048, 512]
VG_OFF = [0, 2048, 4096, 6144]

LAST_RESULT = None  # test harness reads profiling info from here
_NC_CACHE = None


def _bc(ap, insert_at, step, count):
    """Insert a broadcast/strided dim into an AP at position insert_at."""
    new = list(ap.ap)
    new.insert(insert_at, [step, count])
    return bass.AP(tensor=ap.tensor, offset=ap.offset, ap=new)


def _build():
    nc = bacc.Bacc("TRN2", target_bir_lowering=False, debug=False, num_devices=NC)

    pw_dt = f8 if USE_FP8 else bf16

    # hidw: [L, B*HC] hidden slice (l, b, h) ++ bridge_w column
    hidw = nc.dram_tensor("hidw", [L, B * HC + 1], bf16, kind="ExternalInput").ap()
    # wx: w_ih slice [kp, kc, g, jp] (3072) ++ x0T [kp, kc, b] (128)
    wx = nc.dram_tensor("wx", [128, KC * 3 * 128 + KC * B], bf16,
                        kind="ExternalInput").ap()
    whhT = nc.dram_tensor("whhT", [128, KC * 3 * 128], bf16,
                          kind="ExternalInput").ap()
    # biasf cols: (b_r, b_z, 2*b_ih_n, b_hh_n, bb)
    biasf = nc.dram_tensor("biasf", [128, 5], f32, kind="ExternalInput").ap()
    pwg = [
        nc.dram_tensor(f"pwg{g}", [128, NPAIR * 2 * VG_W[g]], pw_dt,
                       kind="ExternalInput").ap()
        for g in range(4)
    ]
    pb256 = nc.dram_tensor("pb256", [1, VC], f32, kind="ExternalInput").ap()
    logp = nc.dram_tensor("logp", [B, VC], f32, kind="ExternalOutput").ap()
    if DEBUG:
        h1dbg = nc.dram_tensor("h1dbg", [128, KC * B], pw_dt,
                               kind="ExternalOutput").ap()
        lgdbg = nc.dram_tensor("lgdbg", [B, VC], f32, kind="ExternalOutput").ap()

    with tile.TileContext(nc) as tc:
        with (
            tc.tile_pool(name="singles", bufs=1) as singles,
            tc.tile_pool(name="gru_ps", bufs=1, space="PSUM") as gru_ps,
            tc.tile_pool(name="proj_ps", bufs=4, space="PSUM") as proj_ps,
            tc.tile_pool(name="expb", bufs=2) as expp,
            tc.tile_pool(name="dram", bufs=1, space="DRAM") as dram,
        ):
            # ---- bulk loads on the Sync HWDGE FIFO -----------------------
            hid_sb = singles.tile([L, B * HC + 1], bf16, tag="hid_sb")
            for a, b_ in ((0, 512), (512, 1024), (1024, 1536), (1536, 2049)):
                nc.sync.dma_start(out=hid_sb[:, a:b_], in_=hidw[:, a:b_])
            wx_sb = singles.tile([128, KC * 3 * 128 + KC * B], bf16, tag="wx_sb")
            nc.sync.dma_start(out=wx_sb, in_=wx)
            whh_sb = singles.tile([128, KC, 3, 128], bf16, tag="whh_sb")
            nc.sync.dma_start(
                out=whh_sb, in_=whhT.rearrange("p (k g j) -> p k g j", k=KC, g=3)
            )
            pwt = [
                singles.tile([128, NPAIR, 2, VG_W[g]], pw_dt, name=f"pwt{g}",
                             tag=f"pwt{g}")
                for g in range(4)
            ]
            # the first collective cannot begin before the ~54us ncfw arming
            # window anyway, so the full weight stream can drain early in
            # parallel with it
            for g in range(4):
                nc.sync.dma_start(
                    out=pwt[g],
                    in_=pwg[g].rearrange("p (q j v) -> p q j v", q=NPAIR, j=2),
                )

            # ---- small loads on the Scalar HWDGE FIFO --------------------
            bias_sb = singles.tile([128, 5], f32, tag="bias_sb")
            nc.scalar.dma_start(out=bias_sb, in_=biasf)
            pbb = singles.tile([B, VC], f32, tag="pbb")
            nc.scalar.dma_start(out=pbb, in_=_bc(pb256[0], 0, 0, B))

            # views into packed tiles
            wih_v = wx_sb[:, 0 : KC * 3 * 128].rearrange(
                "p (k g j) -> p k g j", k=KC, g=3)
            x0T_v = wx_sb[:, KC * 3 * 128 :].rearrange("p (k b) -> p k b", k=KC)
            hid_v = hid_sb[:, 0 : B * HC].rearrange("l (b h) -> l b h", b=B)
            bw_v = hid_sb[:, B * HC : B * HC + 1]

            # ---- PE warmup + ACT sigmoid table preload -------------------
            zc = singles.tile([128, 1], bf16, tag="zc")
            nc.vector.memset(zc, 0.0)
            z512 = singles.tile([128, 512], bf16, tag="z512")
            nc.vector.memset(z512, 0.0)
            zf = singles.tile([128, 1], f32, tag="zf")
            nc.vector.memset(zf, 0.0)
            zf2 = singles.tile([128, 1], f32, tag="zf2")
            nc.scalar.activation(out=zf2, in_=zf, func=FX.Sigmoid)  # table load
            warm_ps = gru_ps.tile([1, 512], f32, tag="warm_ps")

            def warm(n):
                for _ in range(n):
                    nc.tensor.matmul(warm_ps[:], zc[:], z512[:], start=True, stop=True)

            warm(6)

            # ---- bridge: h0T[h,b] = sum_l hid[l,b,h]*bw[l] (own slice) ---
            gru_all = gru_ps.tile([128, 5, B], f32, tag="gru_all")
            h0T_ps = gru_all[:, 0, :]
            grz_ps = gru_all[:, 1:3, :]
            gin_ps = gru_all[:, 3, :]
            ghn_ps = gru_all[:, 4, :]
            for b in range(B):
                nc.tensor.matmul(
                    h0T_ps[:, b : b + 1], hid_v[:, b, :], bw_v,
                    start=True, stop=True,
                )
            h0T_f = singles.tile([HC, B], f32, tag="h0T_f")
            nc.vector.tensor_scalar_add(h0T_f[:], h0T_ps, bias_sb[:, 4:5])
            h0T_bf = singles.tile([HC, B], bf16, tag="h0T_bf")
            nc.vector.tensor_copy(h0T_bf[:], h0T_f[:])


            # ---- AllGather h0 (4 KB) on the GpSimd queue -----------------
            cc1_in = dram.tile([HC, B], bf16, tag="cc1_in")
            cc1_out = dram.tile([NC * HC, B], bf16, tag="cc1_out")
            nc.gpsimd.dma_start(out=cc1_in[:], in_=h0T_bf[:])
            nc.gpsimd.collective_compute(
                "AllGather", ALU.bypass,
                replica_groups=[list(range(NC))],
                ins=[cc1_in.opt()], outs=[cc1_out.opt()],
            )
            # contiguous gather-back: h0full[p, kc, b] = h0[8p+kc, b]
            h0full = singles.tile([128, KC, B], bf16, tag="h0full")
            nc.gpsimd.dma_start(
                out=h0full, in_=cc1_out.opt().rearrange("(p k) b -> p k b", p=128)
            )

            # ---- gate pre-activations for own 128 j-rows -----------------
            # r/z: gi and gh accumulate into one PSUM group (gi during the
            # AllGather, gh after).  n-gate halves stay separate.
            for g in range(2):
                for kc in range(KC):
                    nc.tensor.matmul(
                        grz_ps[:, g, :], wih_v[:, kc, g, :], x0T_v[:, kc, :],
                        start=(kc == 0), stop=False,
                    )
            for kc in range(KC):
                nc.tensor.matmul(
                    gin_ps, wih_v[:, kc, 2, :], x0T_v[:, kc, :],
                    start=(kc == 0), stop=(kc == KC - 1),
                )
            warm(8)
            for g in range(2):
                for kc in range(KC):
                    nc.tensor.matmul(
                        grz_ps[:, g, :], whh_sb[:, kc, g, :], h0full[:, kc, :],
                        start=False, stop=(kc == KC - 1),
                    )
            for kc in range(KC):
                nc.tensor.matmul(
                    ghn_ps, whh_sb[:, kc, 2, :], h0full[:, kc, :],
                    start=(kc == 0), stop=(kc == KC - 1),
                )
            warm(12)

            # ---- gates + h1 (T layout, [128, B]) -------------------------
            rT = singles.tile([128, B], f32, tag="rT")
            nc.scalar.activation(
                out=rT, in_=grz_ps[:, 0, :], func=FX.Sigmoid, bias=bias_sb[:, 0:1]
            )
            zT = singles.tile([128, B], f32, tag="zT")
            nc.scalar.activation(
                out=zT, in_=grz_ps[:, 1, :], func=FX.Sigmoid, bias=bias_sb[:, 1:2]
            )
            hn = singles.tile([128, B], f32, tag="hn")
            nc.vector.tensor_scalar_add(hn[:], ghn_ps, bias_sb[:, 3:4])
            nc.vector.tensor_mul(hn[:], hn[:], rT[:])
            nc.vector.tensor_add(hn[:], hn[:], gin_ps)
            # n = tanh(pre + b_in) = 2*sigmoid(2*pre + 2*b_in) - 1 (no ACT
            # table swap; bias col2 is pre-doubled on host)
            sT = singles.tile([128, B], f32, tag="sT")
            nc.scalar.activation(
                out=sT, in_=hn, func=FX.Sigmoid, scale=2.0, bias=bias_sb[:, 2:3]
            )
            # h1 = n + z*(h0 - n);  u0 = 1-2s = -n
            u0 = singles.tile([128, B], f32, tag="u0")
            nc.vector.tensor_scalar(
                out=u0[:], in0=sT[:], scalar1=-2.0, scalar2=1.0,
                op0=ALU.mult, op1=ALU.add,
            )
            u1 = singles.tile([128, B], f32, tag="u1")
            nc.vector.tensor_add(u1[:], h0T_f[:], u0[:])     # h0 - n
            nc.vector.tensor_mul(u1[:], u1[:], zT[:])        # z*(h0-n)
            h1T = singles.tile([128, B], f32, tag="h1T")
            nc.vector.tensor_sub(h1T[:], u1[:], u0[:])       # + n
            h1q = singles.tile([HC, B], pw_dt, tag="h1q")
            nc.vector.tensor_scalar_mul(h1q[:], h1T[:], H_SCALE if USE_FP8 else 1.0)

            # ---- AllGather h1 (2 KB) -------------------------------------
            cc2_in = dram.tile([HC, B], pw_dt, tag="cc2_in")
            cc2_out = dram.tile([NC * HC, B], pw_dt, tag="cc2_out")
            nc.gpsimd.dma_start(out=cc2_in[:], in_=h1q[:])
            # preload Exp table during the h1 AllGather
            nc.scalar.activation(out=zf2, in_=zf, func=FX.Exp)
            nc.gpsimd.collective_compute(
                "AllGather", ALU.bypass,
                replica_groups=[list(range(NC))],
                ins=[cc2_in.opt()], outs=[cc2_out.opt()],
            )
            # h1full[p, j, b] = h1[8p+j, b]
            h1full = singles.tile([128, KC, B], pw_dt, tag="h1full")
            nc.gpsimd.dma_start(
                out=h1full, in_=cc2_out.opt().rearrange("(p k) b -> p k b", p=128)
            )
            if DEBUG:
                nc.scalar.dma_start(out=h1dbg, in_=h1full[:])
            warm(16)

            # ---- projection: PSUM = 256 * logits, chunked ----------------
            logits = singles.tile([B, VC], f32, tag="logits")
            csums = singles.tile([B, 7], f32, tag="csums")
            for ch in range(N_CH):
                col = ch * 512
                g = min(col // 2048, 3)
                sub = col - VG_OFF[g]
                lg = proj_ps.tile([B, 512], f32, tag="lg")
                if USE_FP8:
                    for q in range(NPAIR):
                        nc.tensor.matmul(
                            lg[:],
                            h1full[:, 2 * q : 2 * q + 2, :],
                            pwt[g][:, q, :, sub : sub + 512],
                            start=(q == 0), stop=(q == NPAIR - 1),
                            perf_mode=DR,
                        )
                else:
                    for kc in range(KC):
                        nc.tensor.matmul(
                            lg[:],
                            h1full[:, kc, :],
                            pwt[g][:, kc // 2, kc % 2, sub : sub + 512],
                            start=(kc == 0), stop=(kc == KC - 1),
                        )
                nc.vector.tensor_add(
                    logits[:, col : col + 512], lg[:], pbb[:, col : col + 512]
                )
                if ch % 2 == 1 or ch == N_CH - 1:
                    ecol = (ch // 2) * 1024
                    ew = col + 512 - ecol
                    eb = expp.tile([B, 1024], bf16, tag="eb")
                    nc.scalar.activation(
                        out=eb[:, :ew], in_=logits[:, ecol : ecol + ew], func=FX.Exp,
                        scale=INV_SCALE if USE_FP8 else 1.0,
                        accum_out=csums[:, ch // 2 : ch // 2 + 1],
                    )
            if DEBUG:
                nc.scalar.dma_start(out=lgdbg, in_=logits[:])

            # ---- global log-sum-exp (AllGather 64 B) ---------------------
            s1 = singles.tile([B, 1], f32, tag="s1")
            nc.vector.reduce_sum(s1, csums[:], axis=AX.X)
            cc3_in = dram.tile([B, 1], f32, tag="cc3_in")
            cc3_out = dram.tile([NC * B, 1], f32, tag="cc3_out")
            nc.gpsimd.dma_start(out=cc3_in[:], in_=s1[:])
            # preload Ln table during the stats AllGather
            nc.scalar.activation(out=zf2, in_=zf, func=FX.Ln)
            nc.gpsimd.collective_compute(
                "AllGather", ALU.bypass,
                replica_groups=[list(range(NC))],
                ins=[cc3_in.opt()], outs=[cc3_out.opt()],
            )
            sAll = singles.tile([B, NC], f32, tag="sAll")
            so = cc3_out[:]
            nc.gpsimd.dma_start(
                out=sAll,
                in_=bass.AP(tensor=so.tensor, offset=so.offset,
                            ap=[[1, B], [B, NC]]),
            )
            gS = singles.tile([B, 1], f32, tag="gS")
            nc.vector.reduce_sum(gS, sAll, axis=AX.X)
            lse = singles.tile([B, 1], f32, tag="lse")
            nc.scalar.activation(out=lse, in_=gS, func=FX.Ln)

            # ---- logp = logits/256 - lse (DVE), write out ----------------
            sc = INV_SCALE if USE_FP8 else 1.0
            for a, b_ in ((0, 2048), (2048, 4608), (4608, VC)):
                nc.vector.tensor_scalar(
                    out=logits[:, a:b_], in0=logits[:, a:b_],
                    scalar1=sc, scalar2=lse[:, 0:1],
                    op0=ALU.mult, op1=ALU.subtract,
                )
                nc.sync.dma_start(out=logp[:, a:b_], in_=logits[:, a:b_])

    nc.compile()
    return nc


def kernel(input, hidden, emb, bridge_w, bridge_b, w_ih, w_hh, b_ih, b_hh,
           proj_w, proj_b):
    global _NC_CACHE, LAST_RESULT
    if _NC_CACHE is None:
        _NC_CACHE = _build()
    nc = _NC_CACHE

    bf = ml_dtypes.bfloat16
    f8np = ml_dtypes.float8_e4m3

    input = np.asarray(input)
    hidden = np.asarray(hidden, dtype=np.float32)
    emb = np.asarray(emb, dtype=np.float32)
    bridge_w = np.asarray(bridge_w, dtype=np.float32)
    bridge_b = np.asarray(bridge_b, dtype=np.float32)
    w_ih = np.asarray(w_ih, dtype=np.float32)
    w_hh = np.asarray(w_hh, dtype=np.float32)
    b_ih = np.asarray(b_ih, dtype=np.float32)
    b_hh = np.asarray(b_hh, dtype=np.float32)
    proj_w = np.asarray(proj_w, dtype=np.float32)
    proj_b = np.asarray(proj_b, dtype=np.float32)

    pw_dt = f8np if USE_FP8 else bf

    # x0T with gates k-map: x0T_pack[p, kc, b] = x0[8p+kc, b]
    x0 = np.maximum(emb[input[:, 0].astype(np.int64)], 0.0)   # [B,H] relu'd
    x0T_pack = x0.T.reshape(128, KC, B)                        # k-major
    bsum = b_ih + b_hh                                         # [3H]
    # per-gate weight views [g, c, jp, k]
    wih_r = w_ih.reshape(3, NC, 128, H)
    whh_r = w_hh.reshape(3, NC, 128, H)

    in_maps = []
    for c in range(NC):
        hs = slice(c * HC, (c + 1) * HC)
        hid_blk = hidden[:, :, hs].transpose(1, 0, 2).reshape(L, B * HC)
        hidw_in = np.concatenate(
            [hid_blk, np.broadcast_to(bridge_w.reshape(L, 1), (L, 1))], axis=1)
        hidw_in = np.ascontiguousarray(hidw_in).astype(bf)

        # [k, g, jp] -> [kp, kc, g, jp] with k = 8*kp + kc
        wih_in = wih_r[:, c].transpose(2, 0, 1).reshape(128, KC, 3, 128)
        whh_in = whh_r[:, c].transpose(2, 0, 1).reshape(128, KC, 3, 128)
        wx_in = np.concatenate(
            [wih_in.reshape(128, KC * 3 * 128), x0T_pack.reshape(128, KC * B)],
            axis=1)
        wx_in = np.ascontiguousarray(wx_in).astype(bf)
        whh_in = np.ascontiguousarray(whh_in.reshape(128, KC * 3 * 128)).astype(bf)

        bias_in = np.stack(
            [bsum[0:H][hs], bsum[H:2 * H][hs],
             2.0 * b_ih[2 * H:3 * H][hs], b_hh[2 * H:3 * H][hs],
             np.full(HC, bridge_b[0], np.float32)], axis=1)
        bias_in = np.ascontiguousarray(bias_in, dtype=np.float32)

        lo, hi = c * VC, min((c + 1) * VC, V)
        pw_blk = proj_w[lo:hi]
        pb_blk = proj_b[lo:hi]
        if hi - lo < VC:
            pad = VC - (hi - lo)
            pw_blk = np.concatenate([pw_blk, np.zeros((pad, H), np.float32)], axis=0)
            pb_blk = np.concatenate([pb_blk, np.full((pad,), PAD_PB, np.float32)])
        # proj k-map: pw_pack[p, q, jj, v] = pwT[8p + 2q + jj, v] (* W_SCALE)
        pwq = pw_blk.T.reshape(128, NPAIR, 2, VC)
        if USE_FP8:
            pwq = pwq * W_SCALE
        pwq = pwq.astype(pw_dt)
        pb_in = np.ascontiguousarray(
            (pb_blk * (W_SCALE * H_SCALE if USE_FP8 else 1.0)).reshape(1, VC),
            dtype=np.float32)

        m = {
            "hidw": hidw_in,
            "wx": wx_in,
            "whhT": whh_in,
            "biasf": bias_in,
            "pb256": pb_in,
        }
        for g in range(4):
            m[f"pwg{g}"] = np.ascontiguousarray(
                pwq[:, :, :, VG_OFF[g] : VG_OFF[g] + VG_W[g]]
            ).reshape(128, NPAIR * 2 * VG_W[g])
        in_maps.append(m)

    res = run_bass_kernel_spmd(nc, in_maps, list(range(NC)))
    LAST_RESULT = res

    logp_full = np.concatenate([res.results[c]["logp"] for c in range(NC)], axis=1)
    logp_full = np.ascontiguousarray(logp_full[:, :V])
    return np.broadcast_to(logp_full[:, None, :], (B, L - 1, V))


# revision 20
# speedup vs baseline: 1.1527x; 1.1527x over previous
"""GRU-decoder kernel for 8 Trainium2 NeuronCores.

Math (all 127 output steps are identical -- see the reference):
    x0   = relu(emb[input[:,0]])                       [B,H]
    h0   = einsum('blh,l->bh', hidden, bridge_w) + bb  [B,H]
    gi   = x0 @ w_ih.T + b_ih ; gh = h0 @ w_hh.T + b_hh
    r,z  = sigmoid(...) ; n = tanh(in + r*hn)
    h1   = (1-z)*n + z*h0
    logp = log_softmax(h1 @ proj_w.T + proj_b)         [B,V]
    out  = broadcast(logp, [B, L-1, V])

Sharding: everything h-sliced.  Core c owns hidden-dim slice
[c*128,(c+1)*128): it computes that slice of h0 exactly (bridge contracts
over L), AllGathers h0 (4 KB), computes the r/z/n gate rows for its slice
exactly (full-H contraction, bf16), forms its h1 slice, and AllGathers h1
(2 KB, fp8).  The projection is vocab-sharded: each core owns V/8 rows of
proj_w stored fp8 (scaled x32) and computes logits with DoubleRow fp8
matmuls (h1 scaled x8, so PSUM = 256*logits).  Softmax needs no
max-subtraction (logits are O(1)); per-core sum(exp) is AllGathered (64 B)
for the global log-sum-exp.

Queueing: bulk weight streams ride the Sync-engine HWDGE FIFO; the
collective chain (payload in / AllGather / gather-back) rides the GpSimd
SWDGE queue so it never waits behind bulk transfers (HWDGE completions are
FIFO per engine).  Gate tanh is computed as 2*sigmoid(2x)-1 so the scalar
engine never swaps activation tables on the critical path; Exp/Ln tables
are preloaded with dummy ops during collective waits.

Contraction index maps (so every gather-back is one contiguous DMA):
  gates:      k = 8*kp + kc   (kp = partition, kc = chunk 0..7)
  projection: k = 8*p + 2*q + jj  (q = DoubleRow pair, jj = subtile)
Host-side packing matches these maps.
"""

import os
import numpy as np
import ml_dtypes

import concourse.bass as bass
import concourse.tile as tile
from concourse import bacc, mybir
from concourse.bass_utils import run_bass_kernel_spmd

B, L, H, V = 16, 128, 1024, 50257
NC = 8
VC = 6656                # per-core vocab shard (13*512); 8*VC = 53248 >= V
HC = 128                 # per-core hidden-dim slice
KC = 8                   # k-chunks of 128 over H
NPAIR = 4                # fp8 DoubleRow k-pairs (256-contraction each)
N_CH = VC // 512         # 13 psum chunks per core

USE_FP8 = True
DEBUG = os.environ.get("KDBG") == "1"
W_SCALE = 32.0           # proj_w pre-scale (host, exact power of 2)
H_SCALE = 8.0            # h1 pre-scale (device, exact power of 2)
INV_SCALE = 1.0 / (W_SCALE * H_SCALE)
PAD_PB = -10000.0        # bias for padded vocab rows -> exp()=0, masked out

f32 = mybir.dt.float32
bf16 = mybir.dt.bfloat16
f8 = mybir.dt.float8e4
FX = mybir.ActivationFunctionType
AX = mybir.AxisListType
ALU = mybir.AluOpType
DR = mybir.MatmulPerfMode.DoubleRow

VG_W = [2048, 2048, 2048, 512]
VG_OFF = [0, 2048, 4096, 6144]

LAST_RESULT = None  # test harness reads profiling info from here
_NC_CACHE = None


def _bc(ap, insert_at, step, count):
    """Insert a broadcast/strided dim into an AP at position insert_at."""
    new = list(ap.ap)
    new.insert(insert_at, [step, count])
    return bass.AP(tensor=ap.tensor, offset=ap.offset, ap=new)


def _build():
    nc = bacc.Bacc("TRN2", target_bir_lowering=False, debug=False, num_devices=NC)

    pw_dt = f8 if USE_FP8 else bf16

    # hidw: [L, B*H] FULL hidden (l, b, kc, hp; own h-chunk first) ++ bw col
    hidw = nc.dram_tensor("hidw", [L, B * H + 1], bf16, kind="ExternalInput").ap()
    # wx: w_ih slice [kp, kc, g, jp] (3072) ++ x0T [kp, kc, b] (128)
    wx = nc.dram_tensor("wx", [128, KC * 3 * 128 + KC * B], bf16,
                        kind="ExternalInput").ap()
    whhT = nc.dram_tensor("whhT", [128, KC * 3 * 128], bf16,
                          kind="ExternalInput").ap()
    # biasf cols: (b_r, b_z, 2*b_ih_n, b_hh_n, bb)
    biasf = nc.dram_tensor("biasf", [128, 5], f32, kind="ExternalInput").ap()
    pwg = [
        nc.dram_tensor(f"pwg{g}", [128, NPAIR * 2 * VG_W[g]], pw_dt,
                       kind="ExternalInput").ap()
        for g in range(4)
    ]
    pb256 = nc.dram_tensor("pb256", [1, VC], f32, kind="ExternalInput").ap()
    logp = nc.dram_tensor("logp", [B, VC], f32, kind="ExternalOutput").ap()
    if DEBUG:
        h1dbg = nc.dram_tensor("h1dbg", [128, KC * B], pw_dt,
                               kind="ExternalOutput").ap()
        lgdbg = nc.dram_tensor("lgdbg", [B, VC], f32, kind="ExternalOutput").ap()

    with tile.TileContext(nc) as tc:
        with (
            tc.tile_pool(name="singles", bufs=1) as singles,
            tc.tile_pool(name="gru_ps", bufs=1, space="PSUM") as gru_ps,
            tc.tile_pool(name="proj_ps", bufs=4, space="PSUM") as proj_ps,
            tc.tile_pool(name="expb", bufs=2) as expp,
            tc.tile_pool(name="dram", bufs=1, space="DRAM") as dram,
        ):
            # ---- bulk loads on the Sync HWDGE FIFO -----------------------
            hid_sb = singles.tile([L, B * H + 1], bf16, tag="hid_sb")
            for a, b_ in ((0, 4096), (4096, 8192), (8192, 12288), (12288, 16385)):
                nc.sync.dma_start(out=hid_sb[:, a:b_], in_=hidw[:, a:b_])
            wx_sb = singles.tile([128, KC * 3 * 128 + KC * B], bf16, tag="wx_sb")
            nc.sync.dma_start(out=wx_sb, in_=wx)
            whh_sb = singles.tile([128, KC, 3, 128], bf16, tag="whh_sb")
            nc.sync.dma_start(
                out=whh_sb, in_=whhT.rearrange("p (k g j) -> p k g j", k=KC, g=3)
            )
            pwt = [
                singles.tile([128, NPAIR, 2, VG_W[g]], pw_dt, name=f"pwt{g}",
                             tag=f"pwt{g}")
                for g in range(4)
            ]
            # the first collective cannot begin before the ~54us ncfw arming
            # window anyway, so the full weight stream can drain early in
            # parallel with it
            for g in range(4):
                nc.sync.dma_start(
                    out=pwt[g],
                    in_=pwg[g].rearrange("p (q j v) -> p q j v", q=NPAIR, j=2),
                )

            # ---- small loads on the Scalar HWDGE FIFO --------------------
            bias_sb = singles.tile([128, 5], f32, tag="bias_sb")
            nc.scalar.dma_start(out=bias_sb, in_=biasf)
            pbb = singles.tile([B, VC], f32, tag="pbb")
            nc.scalar.dma_start(out=pbb, in_=_bc(pb256[0], 0, 0, B))

            # views into packed tiles
            wih_v = wx_sb[:, 0 : KC * 3 * 128].rearrange(
                "p (k g j) -> p k g j", k=KC, g=3)
            x0T_v = wx_sb[:, KC * 3 * 128 :].rearrange("p (k b) -> p k b", k=KC)
            hid_v = hid_sb[:, 0 : B * H].rearrange(
                "l (b k h) -> l b k h", b=B, k=KC)
            bw_v = hid_sb[:, B * H : B * H + 1]

            # ---- PE warmup + ACT sigmoid table preload -------------------
            zc = singles.tile([128, 1], bf16, tag="zc")
            nc.vector.memset(zc, 0.0)
            z512 = singles.tile([128, 512], bf16, tag="z512")
            nc.vector.memset(z512, 0.0)
            zf = singles.tile([128, 1], f32, tag="zf")
            nc.vector.memset(zf, 0.0)
            zf2 = singles.tile([128, 1], f32, tag="zf2")
            nc.scalar.activation(out=zf2, in_=zf, func=FX.Sigmoid)  # table load
            warm_ps = gru_ps.tile([1, 512], f32, tag="warm_ps")

            def warm(n):
                for _ in range(n):
                    nc.tensor.matmul(warm_ps[:], zc[:], z512[:], start=True, stop=True)

            warm(6)

            # ---- bridge: FULL h0 computed redundantly on every core ------
            # (replaces the h0 AllGather; the full hidden streams inside the
            # ~54us collectives-arming window for free)
            h0_ps = gru_ps.tile([128, KC, B], f32, tag="h0_ps")
            gts = gru_ps.tile([128, 4, B], f32, tag="gts")
            grz_ps = gts[:, 0:2, :]
            gin_ps = gts[:, 2, :]
            ghn_ps = gts[:, 3, :]
            for b in range(B):
                for hc in range(KC):
                    nc.tensor.matmul(
                        h0_ps[:, hc, b : b + 1], hid_v[:, b, hc, :], bw_v,
                        start=True, stop=True,
                    )
            h0f_sb = singles.tile([128, KC, B], bf16, tag="h0f_sb")
            nc.vector.tensor_scalar_add(h0f_sb[:], h0_ps[:], bias_sb[:, 4:5])
            # chunk 0 of the per-core packing is the core's OWN h-slice
            h0T_f = singles.tile([HC, B], f32, tag="h0T_f")
            nc.vector.tensor_scalar_add(h0T_f[:], h0_ps[:, 0, :], bias_sb[:, 4:5])

            # ---- gate pre-activations for own 128 j-rows -----------------
            # r/z: gi and gh accumulate into one PSUM group (gi during the
            # AllGather, gh after).  n-gate halves stay separate.
            for g in range(2):
                for kc in range(KC):
                    nc.tensor.matmul(
                        grz_ps[:, g, :], wih_v[:, kc, g, :], x0T_v[:, kc, :],
                        start=(kc == 0), stop=False,
                    )
            for kc in range(KC):
                nc.tensor.matmul(
                    gin_ps, wih_v[:, kc, 2, :], x0T_v[:, kc, :],
                    start=(kc == 0), stop=(kc == KC - 1),
                )
            warm(8)
            for g in range(2):
                for kc in range(KC):
                    nc.tensor.matmul(
                        grz_ps[:, g, :], whh_sb[:, kc, g, :], h0f_sb[:, kc, :],
                        start=False, stop=(kc == KC - 1),
                    )
            for kc in range(KC):
                nc.tensor.matmul(
                    ghn_ps, whh_sb[:, kc, 2, :], h0f_sb[:, kc, :],
                    start=(kc == 0), stop=(kc == KC - 1),
                )
            warm(12)

            # ---- gates + h1 (T layout, [128, B]) -------------------------
            rT = singles.tile([128, B], f32, tag="rT")
            nc.scalar.activation(
                out=rT, in_=grz_ps[:, 0, :], func=FX.Sigmoid, bias=bias_sb[:, 0:1]
            )
            zT = singles.tile([128, B], f32, tag="zT")
            nc.scalar.activation(
                out=zT, in_=grz_ps[:, 1, :], func=FX.Sigmoid, bias=bias_sb[:, 1:2]
            )
            hn = singles.tile([128, B], f32, tag="hn")
            nc.vector.tensor_scalar_add(hn[:], ghn_ps, bias_sb[:, 3:4])
            nc.vector.tensor_mul(hn[:], hn[:], rT[:])
            nc.vector.tensor_add(hn[:], hn[:], gin_ps)
            # n = tanh(pre + b_in) = 2*sigmoid(2*pre + 2*b_in) - 1 (no ACT
            # table swap; bias col2 is pre-doubled on host)
            sT = singles.tile([128, B], f32, tag="sT")
            nc.scalar.activation(
                out=sT, in_=hn, func=FX.Sigmoid, scale=2.0, bias=bias_sb[:, 2:3]
            )
            # h1 = n + z*(h0 - n);  u0 = 1-2s = -n
            u0 = singles.tile([128, B], f32, tag="u0")
            nc.vector.tensor_scalar(
                out=u0[:], in0=sT[:], scalar1=-2.0, scalar2=1.0,
                op0=ALU.mult, op1=ALU.add,
            )
            u1 = singles.tile([128, B], f32, tag="u1")
            nc.vector.tensor_add(u1[:], h0T_f[:], u0[:])     # h0 - n
            nc.vector.tensor_mul(u1[:], u1[:], zT[:])        # z*(h0-n)
            h1T = singles.tile([128, B], f32, tag="h1T")
            nc.vector.tensor_sub(h1T[:], u1[:], u0[:])       # + n
            h1q = singles.tile([HC, B], pw_dt, tag="h1q")
            nc.vector.tensor_scalar_mul(h1q[:], h1T[:], H_SCALE if USE_FP8 else 1.0)

            # ---- AllGather h1 (2 KB) -------------------------------------
            cc2_in = dram.tile([HC, B], pw_dt, tag="cc2_in")
            cc2_out = dram.tile([NC * HC, B], pw_dt, tag="cc2_out")
            nc.gpsimd.dma_start(out=cc2_in[:], in_=h1q[:])
            # preload Exp table during the h1 AllGather
            nc.scalar.activation(out=zf2, in_=zf, func=FX.Exp)
            nc.gpsimd.collective_compute(
                "AllGather", ALU.bypass,
                replica_groups=[list(range(NC))],
                ins=[cc2_in.opt()], outs=[cc2_out.opt()],
            )
            # h1full[p, j, b] = h1[8p+j, b]
            h1full = singles.tile([128, KC, B], pw_dt, tag="h1full")
            nc.gpsimd.dma_start(
                out=h1full, in_=cc2_out.opt().rearrange("(p k) b -> p k b", p=128)
            )
            if DEBUG:
                nc.scalar.dma_start(out=h1dbg, in_=h1full[:])
            warm(16)

            # ---- projection: PSUM = 256 * logits, chunked ----------------
            logits = singles.tile([B, VC], f32, tag="logits")
            csums = singles.tile([B, 7], f32, tag="csums")
            for ch in range(N_CH):
                col = ch * 512
                g = min(col // 2048, 3)
                sub = col - VG_OFF[g]
                lg = proj_ps.tile([B, 512], f32, tag="lg")
                if USE_FP8:
                    for q in range(NPAIR):
                        nc.tensor.matmul(
                            lg[:],
                            h1full[:, 2 * q : 2 * q + 2, :],
                            pwt[g][:, q, :, sub : sub + 512],
                            start=(q == 0), stop=(q == NPAIR - 1),
                            perf_mode=DR,
                        )
                else:
                    for kc in range(KC):
                        nc.tensor.matmul(
                            lg[:],
                            h1full[:, kc, :],
                            pwt[g][:, kc // 2, kc % 2, sub : sub + 512],
                            start=(kc == 0), stop=(kc == KC - 1),
                        )
                nc.vector.tensor_add(
                    logits[:, col : col + 512], lg[:], pbb[:, col : col + 512]
                )
                if ch % 2 == 1 or ch == N_CH - 1:
                    ecol = (ch // 2) * 1024
                    ew = col + 512 - ecol
                    eb = expp.tile([B, 1024], bf16, tag="eb")
                    nc.scalar.activation(
                        out=eb[:, :ew], in_=logits[:, ecol : ecol + ew], func=FX.Exp,
                        scale=INV_SCALE if USE_FP8 else 1.0,
                        accum_out=csums[:, ch // 2 : ch // 2 + 1],
                    )
            if DEBUG:
                nc.scalar.dma_start(out=lgdbg, in_=logits[:])

            # ---- global log-sum-exp (AllGather 64 B) ---------------------
            s1 = singles.tile([B, 1], f32, tag="s1")
            nc.vector.reduce_sum(s1, csums[:], axis=AX.X)
            cc3_in = dram.tile([B, 1], f32, tag="cc3_in")
            cc3_out = dram.tile([NC * B, 1], f32, tag="cc3_out")
            nc.gpsimd.dma_start(out=cc3_in[:], in_=s1[:])
            # preload Ln table during the stats AllGather
            nc.scalar.activation(out=zf2, in_=zf, func=FX.Ln)
            nc.gpsimd.collective_compute(
                "AllGather", ALU.bypass,
                replica_groups=[list(range(NC))],
                ins=[cc3_in.opt()], outs=[cc3_out.opt()],
            )
            sAll = singles.tile([B, NC], f32, tag="sAll")
            so = cc3_out[:]
            nc.gpsimd.dma_start(
                out=sAll,
                in_=bass.AP(tensor=so.tensor, offset=so.offset,
                            ap=[[1, B], [B, NC]]),
            )
            gS = singles.tile([B, 1], f32, tag="gS")
            nc.vector.reduce_sum(gS, sAll, axis=AX.X)
            lse = singles.tile([B, 1], f32, tag="lse")
            nc.scalar.activation(out=lse, in_=gS, func=FX.Ln)

            # ---- logp = logits/256 - lse (DVE), write out ----------------
            sc = INV_SCALE if USE_FP8 else 1.0
            for a, b_ in ((0, 2048), (2048, 4608), (4608, VC)):
                nc.vector.tensor_scalar(
                    out=logits[:, a:b_], in0=logits[:, a:b_],
                    scalar1=sc, scalar2=lse[:, 0:1],
                    op0=ALU.mult, op1=ALU.subtract,
                )
                nc.sync.dma_start(out=logp[:, a:b_], in_=logits[:, a:b_])

    nc.compile()
    return nc


def kernel(input, hidden, emb, bridge_w, bridge_b, w_ih, w_hh, b_ih, b_hh,
           proj_w, proj_b):
    global _NC_CACHE, LAST_RESULT
    if _NC_CACHE is None:
        _NC_CACHE = _build()
    nc = _NC_CACHE

    bf = ml_dtypes.bfloat16
    f8np = ml_dtypes.float8_e4m3

    input = np.asarray(input)
    hidden = np.asarray(hidden, dtype=np.float32)
    emb = np.asarray(emb, dtype=np.float32)
    bridge_w = np.asarray(bridge_w, dtype=np.float32)
    bridge_b = np.asarray(bridge_b, dtype=np.float32)
    w_ih = np.asarray(w_ih, dtype=np.float32)
    w_hh = np.asarray(w_hh, dtype=np.float32)
    b_ih = np.asarray(b_ih, dtype=np.float32)
    b_hh = np.asarray(b_hh, dtype=np.float32)
    proj_w = np.asarray(proj_w, dtype=np.float32)
    proj_b = np.asarray(proj_b, dtype=np.float32)

    pw_dt = f8np if USE_FP8 else bf

    x0 = np.maximum(emb[input[:, 0].astype(np.int64)], 0.0)   # [B,H] relu'd
    x0T_c = x0.T.reshape(KC, 128, B)                           # [kc_glob, p, b]
    hid_t = hidden.transpose(1, 0, 2).reshape(L, B, KC, 128)   # [l, b, kc, hp]
    bsum = b_ih + b_hh                                         # [3H]
    # per-gate weight views [g, cblk, jp, kc_glob, kp]
    wih_r = w_ih.reshape(3, NC, 128, KC, 128)
    whh_r = w_hh.reshape(3, NC, 128, KC, 128)

    in_maps = []
    for c in range(NC):
        # per-core h-chunk order: own chunk first (contraction is
        # order-invariant as long as w/x0/hidden packing agree)
        chunks = [c] + [x for x in range(NC) if x != c]
        hid_blk = hid_t[:, :, chunks, :].reshape(L, B * H)
        hidw_in = np.concatenate(
            [hid_blk, np.broadcast_to(bridge_w.reshape(L, 1), (L, 1))], axis=1)
        hidw_in = np.ascontiguousarray(hidw_in).astype(bf)

        hs = slice(c * HC, (c + 1) * HC)
        x0T_pack = x0T_c[chunks].transpose(1, 0, 2)            # [p, kci, b]
        # [g, jp, kc_glob, kp] -> [kp, kci, g, jp]
        wih_in = wih_r[:, c][:, :, chunks, :].transpose(3, 2, 0, 1)
        whh_in = whh_r[:, c][:, :, chunks, :].transpose(3, 2, 0, 1)
        wx_in = np.concatenate(
            [wih_in.reshape(128, KC * 3 * 128), x0T_pack.reshape(128, KC * B)],
            axis=1)
        wx_in = np.ascontiguousarray(wx_in).astype(bf)
        whh_in = np.ascontiguousarray(whh_in.reshape(128, KC * 3 * 128)).astype(bf)

        bias_in = np.stack(
            [bsum[0:H][hs], bsum[H:2 * H][hs],
             2.0 * b_ih[2 * H:3 * H][hs], b_hh[2 * H:3 * H][hs],
             np.full(HC, bridge_b[0], np.float32)], axis=1)
        bias_in = np.ascontiguousarray(bias_in, dtype=np.float32)

        lo, hi = c * VC, min((c + 1) * VC, V)
        pw_blk = proj_w[lo:hi]
        pb_blk = proj_b[lo:hi]
        if hi - lo < VC:
            pad = VC - (hi - lo)
            pw_blk = np.concatenate([pw_blk, np.zeros((pad, H), np.float32)], axis=0)
            pb_blk = np.concatenate([pb_blk, np.full((pad,), PAD_PB, np.float32)])
        # proj k-map: pw_pack[p, q, jj, v] = pwT[8p + 2q + jj, v] (* W_SCALE)
        pwq = pw_blk.T.reshape(128, NPAIR, 2, VC)
        if USE_FP8:
            pwq = pwq * W_SCALE
        pwq = pwq.astype(pw_dt)
        pb_in = np.ascontiguousarray(
            (pb_blk * (W_SCALE * H_SCALE if USE_FP8 else 1.0)).reshape(1, VC),
            dtype=np.float32)

        m = {
            "hidw": hidw_in,
            "wx": wx_in,
            "whhT": whh_in,
            "biasf": bias_in,
            "pb256": pb_in,
        }
        for g in range(4):
            m[f"pwg{g}"] = np.ascontiguousarray(
                pwq[:, :, :, VG_OFF[g] : VG_OFF[g] + VG_W[g]]
            ).reshape(128, NPAIR * 2 * VG_W[g])
        in_maps.append(m)

    res = run_bass_kernel_spmd(nc, in_maps, list(range(NC)))
    LAST_RESULT = res

    logp_full = np.concatenate([res.results[c]["logp"] for c in range(NC)], axis=1)
    logp_full = np.ascontiguousarray(logp_full[:, :V])
    return np.broadcast_to(logp_full[:, None, :], (B, L - 1, V))


# revision 22
# speedup vs baseline: 1.2736x; 1.1049x over previous
"""GRU-decoder kernel for 8 Trainium2 NeuronCores.

Math (all 127 output steps are identical -- see the reference):
    x0   = relu(emb[input[:,0]])                       [B,H]
    h0   = einsum('blh,l->bh', hidden, bridge_w) + bb  [B,H]
    gi   = x0 @ w_ih.T + b_ih ; gh = h0 @ w_hh.T + b_hh
    r,z  = sigmoid(...) ; n = tanh(in + r*hn)
    h1   = (1-z)*n + z*h0
    logp = log_softmax(h1 @ proj_w.T + proj_b)         [B,V]
    out  = broadcast(logp, [B, L-1, V])

Sharding: everything h-sliced.  Core c owns hidden-dim slice
[c*128,(c+1)*128): it computes that slice of h0 exactly (bridge contracts
over L), AllGathers h0 (4 KB), computes the r/z/n gate rows for its slice
exactly (full-H contraction, bf16), forms its h1 slice, and AllGathers h1
(2 KB, fp8).  The projection is vocab-sharded: each core owns V/8 rows of
proj_w stored fp8 (scaled x32) and computes logits with DoubleRow fp8
matmuls (h1 scaled x8, so PSUM = 256*logits).  Softmax needs no
max-subtraction (logits are O(1)); per-core sum(exp) is AllGathered (64 B)
for the global log-sum-exp.

Queueing: bulk weight streams ride the Sync-engine HWDGE FIFO; the
collective chain (payload in / AllGather / gather-back) rides the GpSimd
SWDGE queue so it never waits behind bulk transfers (HWDGE completions are
FIFO per engine).  Gate tanh is computed as 2*sigmoid(2x)-1 so the scalar
engine never swaps activation tables on the critical path; Exp/Ln tables
are preloaded with dummy ops during collective waits.

Contraction index maps (so every gather-back is one contiguous DMA):
  gates:      k = 8*kp + kc   (kp = partition, kc = chunk 0..7)
  projection: k = 8*p + 2*q + jj  (q = DoubleRow pair, jj = subtile)
Host-side packing matches these maps.
"""

import os
import numpy as np
import ml_dtypes

import concourse.bass as bass
import concourse.tile as tile
from concourse import bacc, mybir
from concourse.bass_utils import run_bass_kernel_spmd

B, L, H, V = 16, 128, 1024, 50257
NC = 8
VC = 6656                # per-core vocab shard (13*512); 8*VC = 53248 >= V
HC = 128                 # per-core hidden-dim slice
KC = 8                   # k-chunks of 128 over H
NPAIR = 4                # fp8 DoubleRow k-pairs (256-contraction each)
N_CH = VC // 512         # 13 psum chunks per core

USE_FP8 = True
DEBUG = os.environ.get("KDBG") == "1"
W_SCALE = 32.0           # proj_w pre-scale (host, exact power of 2)
H_SCALE = 8.0            # h1 pre-scale (device, exact power of 2)
INV_SCALE = 1.0 / (W_SCALE * H_SCALE)
PAD_PB = -10000.0        # bias for padded vocab rows -> exp()=0, masked out

f32 = mybir.dt.float32
bf16 = mybir.dt.bfloat16
f8 = mybir.dt.float8e4
FX = mybir.ActivationFunctionType
AX = mybir.AxisListType
ALU = mybir.AluOpType
DR = mybir.MatmulPerfMode.DoubleRow

VG_W = [2048, 2048, 2048, 512]
VG_OFF = [0, 2048, 4096, 6144]

LAST_RESULT = None  # test harness reads profiling info from here
_NC_CACHE = None


def _bc(ap, insert_at, step, count):
    """Insert a broadcast/strided dim into an AP at position insert_at."""
    new = list(ap.ap)
    new.insert(insert_at, [step, count])
    return bass.AP(tensor=ap.tensor, offset=ap.offset, ap=new)


def _build():
    nc = bacc.Bacc("TRN2", target_bir_lowering=False, debug=False, num_devices=NC)

    pw_dt = f8 if USE_FP8 else bf16

    # hidw: [L, B*H] FULL hidden (l, b, kc, hp; own h-chunk first) ++ bw col
    hidw = nc.dram_tensor("hidw", [L, B * H + 1], bf16, kind="ExternalInput").ap()
    # wx: w_ih slice [kp, kc, g, jp] (3072) ++ x0T [kp, kc, b] (128)
    wx = nc.dram_tensor("wx", [128, KC * 3 * 128 + KC * B], bf16,
                        kind="ExternalInput").ap()
    whhT = nc.dram_tensor("whhT", [128, KC * 3 * 128], bf16,
                          kind="ExternalInput").ap()
    # biasf cols: (b_r, b_z, 2*b_ih_n, b_hh_n, bb)
    biasf = nc.dram_tensor("biasf", [128, 5], f32, kind="ExternalInput").ap()
    pwg = [
        nc.dram_tensor(f"pwg{g}", [128, NPAIR * 2 * VG_W[g]], pw_dt,
                       kind="ExternalInput").ap()
        for g in range(4)
    ]
    pb256 = nc.dram_tensor("pb256", [1, VC], f32, kind="ExternalInput").ap()
    logp = nc.dram_tensor("logp", [B, VC], f32, kind="ExternalOutput").ap()
    if DEBUG:
        h1dbg = nc.dram_tensor("h1dbg", [128, KC * B], pw_dt,
                               kind="ExternalOutput").ap()
        lgdbg = nc.dram_tensor("lgdbg", [B, VC], f32, kind="ExternalOutput").ap()

    with tile.TileContext(nc) as tc:
        with (
            tc.tile_pool(name="singles", bufs=1) as singles,
            tc.tile_pool(name="gru_ps", bufs=1, space="PSUM") as gru_ps,
            tc.tile_pool(name="proj_ps", bufs=4, space="PSUM") as proj_ps,
            tc.tile_pool(name="expb", bufs=2) as expp,
            tc.tile_pool(name="dram", bufs=1, space="DRAM") as dram,
        ):
            # ---- bulk loads on the Sync HWDGE FIFO -----------------------
            hid_sb = singles.tile([L, B * H + 1], bf16, tag="hid_sb")
            for a, b_ in ((0, 4096), (4096, 8192), (8192, 12288), (12288, 16385)):
                nc.sync.dma_start(out=hid_sb[:, a:b_], in_=hidw[:, a:b_])
            wx_sb = singles.tile([128, KC * 3 * 128 + KC * B], bf16, tag="wx_sb")
            nc.sync.dma_start(out=wx_sb, in_=wx)
            whh_sb = singles.tile([128, KC, 3, 128], bf16, tag="whh_sb")
            nc.sync.dma_start(
                out=whh_sb, in_=whhT.rearrange("p (k g j) -> p k g j", k=KC, g=3)
            )
            pwt = [
                singles.tile([128, NPAIR, 2, VG_W[g]], pw_dt, name=f"pwt{g}",
                             tag=f"pwt{g}")
                for g in range(4)
            ]
            # the first collective cannot begin before the ~54us ncfw arming
            # window anyway, so the full weight stream can drain early in
            # parallel with it
            for g in range(4):
                nc.sync.dma_start(
                    out=pwt[g],
                    in_=pwg[g].rearrange("p (q j v) -> p q j v", q=NPAIR, j=2),
                )

            # ---- small loads on the Scalar HWDGE FIFO --------------------
            bias_sb = singles.tile([128, 5], f32, tag="bias_sb")
            nc.scalar.dma_start(out=bias_sb, in_=biasf)
            pbb = singles.tile([B, VC], f32, tag="pbb")
            nc.scalar.dma_start(out=pbb, in_=_bc(pb256[0], 0, 0, B))

            # views into packed tiles
            wih_v = wx_sb[:, 0 : KC * 3 * 128].rearrange(
                "p (k g j) -> p k g j", k=KC, g=3)
            x0T_v = wx_sb[:, KC * 3 * 128 :].rearrange("p (k b) -> p k b", k=KC)
            hid_v = hid_sb[:, 0 : B * H].rearrange(
                "l (b k h) -> l b k h", b=B, k=KC)
            bw_v = hid_sb[:, B * H : B * H + 1]

            # ---- PE warmup + ACT sigmoid table preload -------------------
            zc = singles.tile([128, 1], bf16, tag="zc")
            nc.vector.memset(zc, 0.0)
            z512 = singles.tile([128, 512], bf16, tag="z512")
            nc.vector.memset(z512, 0.0)
            zf = singles.tile([128, 1], f32, tag="zf")
            nc.vector.memset(zf, 0.0)
            zf2 = singles.tile([128, 1], f32, tag="zf2")
            nc.scalar.activation(out=zf2, in_=zf, func=FX.Sigmoid)  # table load
            warm_ps = gru_ps.tile([1, 512], f32, tag="warm_ps")

            def warm(n):
                for _ in range(n):
                    nc.tensor.matmul(warm_ps[:], zc[:], z512[:], start=True, stop=True)

            warm(4)

            # ---- bridge: FULL h0 computed redundantly on every core ------
            # (replaces the h0 AllGather; the full hidden streams inside the
            # ~54us collectives-arming window for free)
            h0_ps = gru_ps.tile([128, KC, B], f32, tag="h0_ps")
            gts = gru_ps.tile([128, 4, B], f32, tag="gts")
            grz_ps = gts[:, 0:2, :]
            gin_ps = gts[:, 2, :]
            ghn_ps = gts[:, 3, :]
            for b in range(B):
                for hc in range(KC):
                    nc.tensor.matmul(
                        h0_ps[:, hc, b : b + 1], hid_v[:, b, hc, :], bw_v,
                        start=True, stop=True,
                    )
            h0f_sb = singles.tile([128, KC, B], bf16, tag="h0f_sb")
            nc.vector.tensor_scalar_add(h0f_sb[:], h0_ps[:], bias_sb[:, 4:5])
            # chunk 0 of the per-core packing is the core's OWN h-slice
            h0T_f = singles.tile([HC, B], f32, tag="h0T_f")
            nc.vector.tensor_scalar_add(h0T_f[:], h0_ps[:, 0, :], bias_sb[:, 4:5])

            # ---- gate pre-activations for own 128 j-rows -----------------
            # r/z: gi and gh accumulate into one PSUM group (gi during the
            # AllGather, gh after).  n-gate halves stay separate.
            for g in range(2):
                for kc in range(KC):
                    nc.tensor.matmul(
                        grz_ps[:, g, :], wih_v[:, kc, g, :], x0T_v[:, kc, :],
                        start=(kc == 0), stop=False,
                    )
            for kc in range(KC):
                nc.tensor.matmul(
                    gin_ps, wih_v[:, kc, 2, :], x0T_v[:, kc, :],
                    start=(kc == 0), stop=(kc == KC - 1),
                )
            for g in range(2):
                for kc in range(KC):
                    nc.tensor.matmul(
                        grz_ps[:, g, :], whh_sb[:, kc, g, :], h0f_sb[:, kc, :],
                        start=False, stop=(kc == KC - 1),
                    )
            for kc in range(KC):
                nc.tensor.matmul(
                    ghn_ps, whh_sb[:, kc, 2, :], h0f_sb[:, kc, :],
                    start=(kc == 0), stop=(kc == KC - 1),
                )
            warm(6)

            # ---- gates + h1 (T layout, [128, B]) -------------------------
            rT = singles.tile([128, B], f32, tag="rT")
            nc.scalar.activation(
                out=rT, in_=grz_ps[:, 0, :], func=FX.Sigmoid, bias=bias_sb[:, 0:1]
            )
            zT = singles.tile([128, B], f32, tag="zT")
            nc.scalar.activation(
                out=zT, in_=grz_ps[:, 1, :], func=FX.Sigmoid, bias=bias_sb[:, 1:2]
            )
            hn = singles.tile([128, B], f32, tag="hn")
            nc.vector.tensor_scalar_add(hn[:], ghn_ps, bias_sb[:, 3:4])
            nc.vector.tensor_mul(hn[:], hn[:], rT[:])
            nc.vector.tensor_add(hn[:], hn[:], gin_ps)
            # n = tanh(pre + b_in) = 2*sigmoid(2*pre + 2*b_in) - 1 (no ACT
            # table swap; bias col2 is pre-doubled on host)
            sT = singles.tile([128, B], f32, tag="sT")
            nc.scalar.activation(
                out=sT, in_=hn, func=FX.Sigmoid, scale=2.0, bias=bias_sb[:, 2:3]
            )
            # h1 = n + z*(h0 - n);  u0 = 1-2s = -n
            u0 = singles.tile([128, B], f32, tag="u0")
            nc.vector.tensor_scalar(
                out=u0[:], in0=sT[:], scalar1=-2.0, scalar2=1.0,
                op0=ALU.mult, op1=ALU.add,
            )
            u1 = singles.tile([128, B], f32, tag="u1")
            nc.vector.tensor_add(u1[:], h0T_f[:], u0[:])     # h0 - n
            nc.vector.tensor_mul(u1[:], u1[:], zT[:])        # z*(h0-n)
            h1T = singles.tile([128, B], f32, tag="h1T")
            nc.vector.tensor_sub(h1T[:], u1[:], u0[:])       # + n
            h1q = singles.tile([HC, B], pw_dt, tag="h1q")
            nc.vector.tensor_scalar_mul(h1q[:], h1T[:], H_SCALE if USE_FP8 else 1.0)

            # ---- AllGather h1 (2 KB) -------------------------------------
            cc2_in = dram.tile([HC, B], pw_dt, tag="cc2_in")
            cc2_out = dram.tile([NC * HC, B], pw_dt, tag="cc2_out")
            nc.gpsimd.dma_start(out=cc2_in[:], in_=h1q[:])
            # preload Exp table during the h1 AllGather
            nc.scalar.activation(out=zf2, in_=zf, func=FX.Exp)
            nc.gpsimd.collective_compute(
                "AllGather", ALU.bypass,
                replica_groups=[list(range(NC))],
                ins=[cc2_in.opt()], outs=[cc2_out.opt()],
            )
            # h1full[p, j, b] = h1[8p+j, b]
            h1full = singles.tile([128, KC, B], pw_dt, tag="h1full")
            nc.gpsimd.dma_start(
                out=h1full, in_=cc2_out.opt().rearrange("(p k) b -> p k b", p=128)
            )
            if DEBUG:
                nc.scalar.dma_start(out=h1dbg, in_=h1full[:])
            # dependent PE warm: ramps the p-state right when h1full lands,
            # immediately before the projection burst
            if USE_FP8:
                for _ in range(4):
                    wps = proj_ps.tile([B, 512], f32, tag="lg")
                    nc.tensor.matmul(
                        wps[:], h1full[:, 0:2, :], pwt[0][:, 0, :, 0:512],
                        start=True, stop=True, perf_mode=DR,
                    )

            # ---- projection: PSUM = 256 * logits, chunked ----------------
            logits = singles.tile([B, VC], f32, tag="logits")
            csums = singles.tile([B, 7], f32, tag="csums")
            for ch in range(N_CH):
                col = ch * 512
                g = min(col // 2048, 3)
                sub = col - VG_OFF[g]
                lg = proj_ps.tile([B, 512], f32, tag="lg")
                if USE_FP8:
                    for q in range(NPAIR):
                        nc.tensor.matmul(
                            lg[:],
                            h1full[:, 2 * q : 2 * q + 2, :],
                            pwt[g][:, q, :, sub : sub + 512],
                            start=(q == 0), stop=(q == NPAIR - 1),
                            perf_mode=DR,
                        )
                else:
                    for kc in range(KC):
                        nc.tensor.matmul(
                            lg[:],
                            h1full[:, kc, :],
                            pwt[g][:, kc // 2, kc % 2, sub : sub + 512],
                            start=(kc == 0), stop=(kc == KC - 1),
                        )
                nc.vector.tensor_add(
                    logits[:, col : col + 512], lg[:], pbb[:, col : col + 512]
                )
                if ch % 2 == 1 or ch == N_CH - 1:
                    ecol = (ch // 2) * 1024
                    ew = col + 512 - ecol
                    eb = expp.tile([B, 1024], bf16, tag="eb")
                    nc.scalar.activation(
                        out=eb[:, :ew], in_=logits[:, ecol : ecol + ew], func=FX.Exp,
                        scale=INV_SCALE if USE_FP8 else 1.0,
                        accum_out=csums[:, ch // 2 : ch // 2 + 1],
                    )
            if DEBUG:
                nc.scalar.dma_start(out=lgdbg, in_=logits[:])

            # ---- global log-sum-exp (AllGather 64 B) ---------------------
            s1 = singles.tile([B, 1], f32, tag="s1")
            nc.vector.reduce_sum(s1, csums[:], axis=AX.X)
            cc3_in = dram.tile([B, 1], f32, tag="cc3_in")
            cc3_out = dram.tile([NC * B, 1], f32, tag="cc3_out")
            nc.gpsimd.dma_start(out=cc3_in[:], in_=s1[:])
            # preload Ln table during the stats AllGather
            nc.scalar.activation(out=zf2, in_=zf, func=FX.Ln)
            nc.gpsimd.collective_compute(
                "AllGather", ALU.bypass,
                replica_groups=[list(range(NC))],
                ins=[cc3_in.opt()], outs=[cc3_out.opt()],
            )
            sAll = singles.tile([B, NC], f32, tag="sAll")
            so = cc3_out[:]
            nc.gpsimd.dma_start(
                out=sAll,
                in_=bass.AP(tensor=so.tensor, offset=so.offset,
                            ap=[[1, B], [B, NC]]),
            )
            gS = singles.tile([B, 1], f32, tag="gS")
            nc.vector.reduce_sum(gS, sAll, axis=AX.X)
            lse = singles.tile([B, 1], f32, tag="lse")
            nc.scalar.activation(out=lse, in_=gS, func=FX.Ln)

            # ---- logp = logits/256 - lse (DVE), write out ----------------
            sc = INV_SCALE if USE_FP8 else 1.0
            for a, b_ in ((0, 1664), (1664, 3328), (3328, 4992), (4992, VC)):
                nc.vector.tensor_scalar(
                    out=logits[:, a:b_], in0=logits[:, a:b_],
                    scalar1=sc, scalar2=lse[:, 0:1],
                    op0=ALU.mult, op1=ALU.subtract,
                )
                nc.sync.dma_start(out=logp[:, a:b_], in_=logits[:, a:b_])

    nc.compile()
    return nc


def kernel(input, hidden, emb, bridge_w, bridge_b, w_ih, w_hh, b_ih, b_hh,
           proj_w, proj_b):
    global _NC_CACHE, LAST_RESULT
    if _NC_CACHE is None:
        _NC_CACHE = _build()
    nc = _NC_CACHE

    bf = ml_dtypes.bfloat16
    f8np = ml_dtypes.float8_e4m3

    input = np.asarray(input)
    hidden = np.asarray(hidden, dtype=np.float32)
    emb = np.asarray(emb, dtype=np.float32)
    bridge_w = np.asarray(bridge_w, dtype=np.float32)
    bridge_b = np.asarray(bridge_b, dtype=np.float32)
    w_ih = np.asarray(w_ih, dtype=np.float32)
    w_hh = np.asarray(w_hh, dtype=np.float32)
    b_ih = np.asarray(b_ih, dtype=np.float32)
    b_hh = np.asarray(b_hh, dtype=np.float32)
    proj_w = np.asarray(proj_w, dtype=np.float32)
    proj_b = np.asarray(proj_b, dtype=np.float32)

    pw_dt = f8np if USE_FP8 else bf

    x0 = np.maximum(emb[input[:, 0].astype(np.int64)], 0.0)   # [B,H] relu'd
    x0T_c = x0.T.reshape(KC, 128, B)                           # [kc_glob, p, b]
    hid_t = hidden.transpose(1, 0, 2).reshape(L, B, KC, 128)   # [l, b, kc, hp]
    bsum = b_ih + b_hh                                         # [3H]
    # per-gate weight views [g, cblk, jp, kc_glob, kp]
    wih_r = w_ih.reshape(3, NC, 128, KC, 128)
    whh_r = w_hh.reshape(3, NC, 128, KC, 128)

    in_maps = []
    for c in range(NC):
        # per-core h-chunk order: own chunk first (contraction is
        # order-invariant as long as w/x0/hidden packing agree)
        chunks = [c] + [x for x in range(NC) if x != c]
        hid_blk = hid_t[:, :, chunks, :].reshape(L, B * H)
        hidw_in = np.concatenate(
            [hid_blk, np.broadcast_to(bridge_w.reshape(L, 1), (L, 1))], axis=1)
        hidw_in = np.ascontiguousarray(hidw_in).astype(bf)

        hs = slice(c * HC, (c + 1) * HC)
        x0T_pack = x0T_c[chunks].transpose(1, 0, 2)            # [p, kci, b]
        # [g, jp, kc_glob, kp] -> [kp, kci, g, jp]
        wih_in = wih_r[:, c][:, :, chunks, :].transpose(3, 2, 0, 1)
        whh_in = whh_r[:, c][:, :, chunks, :].transpose(3, 2, 0, 1)
        wx_in = np.concatenate(
            [wih_in.reshape(128, KC * 3 * 128), x0T_pack.reshape(128, KC * B)],
            axis=1)
        wx_in = np.ascontiguousarray(wx_in).astype(bf)
        whh_in = np.ascontiguousarray(whh_in.reshape(128, KC * 3 * 128)).astype(bf)

        bias_in = np.stack(
            [bsum[0:H][hs], bsum[H:2 * H][hs],
             2.0 * b_ih[2 * H:3 * H][hs], b_hh[2 * H:3 * H][hs],
             np.full(HC, bridge_b[0], np.float32)], axis=1)
        bias_in = np.ascontiguousarray(bias_in, dtype=np.float32)

        lo, hi = c * VC, min((c + 1) * VC, V)
        pw_blk = proj_w[lo:hi]
        pb_blk = proj_b[lo:hi]
        if hi - lo < VC:
            pad = VC - (hi - lo)
            pw_blk = np.concatenate([pw_blk, np.zeros((pad, H), np.float32)], axis=0)
            pb_blk = np.concatenate([pb_blk, np.full((pad,), PAD_PB, np.float32)])
        # proj k-map: pw_pack[p, q, jj, v] = pwT[8p + 2q + jj, v] (* W_SCALE)
        pwq = pw_blk.T.reshape(128, NPAIR, 2, VC)
        if USE_FP8:
            pwq = pwq * W_SCALE
        pwq = pwq.astype(pw_dt)
        pb_in = np.ascontiguousarray(
            (pb_blk * (W_SCALE * H_SCALE if USE_FP8 else 1.0)).reshape(1, VC),
            dtype=np.float32)

        m = {
            "hidw": hidw_in,
            "wx": wx_in,
            "whhT": whh_in,
            "biasf": bias_in,
            "pb256": pb_in,
        }
        for g in range(4):
            m[f"pwg{g}"] = np.ascontiguousarray(
                pwq[:, :, :, VG_OFF[g] : VG_OFF[g] + VG_W[g]]
            ).reshape(128, NPAIR * 2 * VG_W[g])
        in_maps.append(m)

    res = run_bass_kernel_spmd(nc, in_maps, list(range(NC)))
    LAST_RESULT = res

    logp_full = np.concatenate([res.results[c]["logp"] for c in range(NC)], axis=1)
    logp_full = np.ascontiguousarray(logp_full[:, :V])
    return np.broadcast_to(logp_full[:, None, :], (B, L - 1, V))
